# revision 20
# baseline (speedup 1.0000x reference)
"""IntraSentenceAttention Trainium2 kernel (v2).

out[b,t,:] = sum_s P[t,s] x[b,s,:],  P = row-normalized exp(x@x^T + min(t-s,10))
masked by m_t, m_s (no max-subtract).

Strategy (8 NeuronCores, data-parallel over batch, 4 batches/core):
  - Scores computed transposed E^T[s,t] so PV needs no transpose. Tiles with
    t_tile <= s_tile-2 underflow to 0 and are skipped: 43 of 64 survive.
  - dist trick: exp(min(t-s,10)) == e^10 everywhere except a 265-diagonal band
    (t-s < 10). Fold e^10 into the ACT exp bias (uniform A = 10 + EXP_BIAS);
    multiply only the band window by a resident [128,265] master tile holding
    exp(min(t-s,10)-10)  (value 1.0 at the window edge, ->0 below diagonal).
  - Region split per s-row: band tiles {s-1,s,s+1} exp'd to fp16; far tiles
    (t_tile >= s_tile+2, where dist==10 exactly) exp'd to fp8e4 and consumed
    by DoubleRow fp8 matmuls (2 s-tiles per pass, 0.5 cyc/col) in PV.
  - exp work split ACT/DVE: most slices on ACT (func=Exp); a tuned share on
    DVE via Schraudolph (i8 = S*8*log2e + b, bitcast int8->fp8e4). The +-3-7%
    elementwise noise averages out across ~512 softmax terms (verified
    end-to-end rel err << 2e-2 tolerance).
  - PV accumulates [128, D+1] per 128-row tile; col D (ones in xm) gives the
    row sum r. Epilogue = ONE DVE tensor_scalar: out = (acc / r) * m_t -> fp16.
    (eps is negligible: r >= ~1e3 with these biases.)
  - All DMAs have >=512B contiguous descriptors via host-side prepacking;
    issue split across SP and Pool queues to avoid sequencer serialization.
"""

import sys

sys.path.insert(0, "/opt/trn_rl_repo")

import numpy as np
import ml_dtypes

import concourse.bacc as bacc
import concourse.tile as tile
from concourse import mybir
from concourse.bass_utils import run_bass_kernel_spmd

B, T, D = 32, 1024, 128
NCORES = 8
BPC = B // NCORES
NT = T // 128
DIST_CAP = 10

F32 = mybir.dt.float32
F16 = mybir.dt.float16
F8 = mybir.dt.float8e4
I16 = mybir.dt.int16
I8 = mybir.dt.int8
DR = mybir.MatmulPerfMode.DoubleRow

LOG2E = float(np.log2(np.e))
C8 = 8.0 * LOG2E          # Schraudolph fp8 scale
SIG8 = -0.46              # Schraudolph shift (centers the log-error)

# exp argument cap: largest score is bounded by max ||x_t||^2 (Cauchy-Schwarz);
# A = 10 + EXP_BIAS chosen so exp(S + A) <= 224 < fp8e4m3 max.
FP8_MAX_ARG = float(np.log(224.0))

# ---- static schedule tables (per batch) -----------------------------------
# s-row widths: row s covers t in [max(0,s-1)*128, 1024)
# Groups: B* produce fp16 band tiles {s-1,s,s+1}; F* produce fp8 far tiles
# (t_tile >= s+2). Each QK chunk: (s, pcol, t0, w), pcol never crossing a
# 512-col PSUM bank boundary.
GROUPS = [
    dict(  # band s=0,1,2 -> E16 flat [128, 1152)
        name="B1", width=1024,
        qk=[(0, 0, 0, 256), (1, 256, 0, 256), (1, 512, 256, 128),
            (2, 640, 128, 384)],
        act=[("E16", 128, 0, 1024)],
        schraud=[],
        band=[0, 1, 2],
    ),
    dict(  # far s=0
        name="F0", width=768,
        qk=[(0, 0, 256, 512), (0, 512, 768, 256)],
        act=[("E8", (0, 256), 0, 768)],
        schraud=[],
        band=[],
    ),
    dict(  # band s=3,4 -> E16 flat [1152, 1920)
        name="B2", width=768,
        qk=[(3, 0, 256, 384), (4, 384, 384, 128), (4, 512, 512, 256)],
        act=[("E16", 1152, 0, 768)],
        schraud=[],
        band=[3, 4],
    ),
    dict(  # far s=1  (Schraudolph on DVE)
        name="F1b", width=640,
        qk=[(1, 0, 384, 512), (1, 512, 896, 128)],
        act=[],
        schraud=[("E8", (1, 384), 0, 640)],
        band=[],
    ),
    dict(  # far s=2,3
        name="F2a", width=896,
        qk=[(2, 0, 512, 512), (3, 512, 640, 384)],
        act=[("E8", (2, 512), 0, 512), ("E8", (3, 640), 512, 384)],
        schraud=[],
        band=[],
    ),
    dict(  # band s=5,6,7 -> E16 flat [1920, 2944)
        name="B3", width=1024,
        qk=[(5, 0, 512, 384), (6, 384, 640, 128), (6, 512, 768, 256),
            (7, 768, 768, 256)],
        act=[("E16", 1920, 0, 1024)],
        schraud=[],
        band=[5, 6, 7],
    ),
    dict(  # far s=4,5  (Schraudolph on DVE)
        name="F2b", width=384,
        qk=[(4, 0, 768, 256), (5, 256, 896, 128)],
        act=[],
        schraud=[("E8", (4, 768), 0, 256), ("E8", (5, 896), 256, 128)],
        band=[],
    ),
]
# PV tiles emitted after each group (deps satisfied at that point)
PV_AFTER = {"B1": [0, 1], "F0": [], "B2": [2], "F1b": [3], "F2a": [],
            "B3": [4, 5], "F2b": [6, 7]}
POOL_BAND = (1, 4, 6)   # band-mul rows offloaded to the (idle) Pool engine
EPI_ACT = ()            # PV tiles whose acc-copy runs on ACT instead of DVE

# band-mul window (E16 flat offset, width) per s: j in [0,265) except edges
def _band_window(s):
    j0 = 128 if s == 0 else 0
    j1 = min(265, T - (s - 1) * 128)
    return s * 384 + j0, j0, j1 - j0  # (flat offset, ed offset, width)


def _build_module(act_bias):
    nc = bacc.Bacc("TRN2", target_bir_lowering=False, debug=False, num_devices=NCORES)
    xT = nc.declare_dram_parameter("xT", [BPC, 128, T], F16, isOutput=False)
    xm8 = nc.declare_dram_parameter("xm8", [BPC, 128, 4, 2, D + 1], F8, isOutput=False)
    xm16 = nc.declare_dram_parameter("xm16", [BPC, 128, NT, D + 1], F16, isOutput=False)
    ed = nc.declare_dram_parameter("ed", [128, 265], F16, isOutput=False)
    # y cols [0,1024) = acc/16 per (tt,d); cols [1024,1040) = r/16 per pair-col
    y = nc.declare_dram_parameter("y", [BPC, 128, T + 2 * NT], F16, isOutput=True)

    Exp = mybir.ActivationFunctionType.Exp
    Copy = mybir.ActivationFunctionType.Copy
    Mult = mybir.AluOpType.mult
    Add = mybir.AluOpType.add
    Div = mybir.AluOpType.divide
    b8 = act_bias * C8 + 56.0 + SIG8

    with tile.TileContext(nc) as tc:
        with (
            tc.tile_pool(name="cst", bufs=1) as cst,
            tc.tile_pool(name="xtp", bufs=3) as xtp,
            tc.tile_pool(name="xmp", bufs=3) as xmp,
            tc.tile_pool(name="e16p", bufs=2) as e16p,
            tc.tile_pool(name="e8p", bufs=2) as e8p,
            tc.tile_pool(name="yp", bufs=2) as yp,
            tc.tile_pool(name="ps_g", bufs=3, space="PSUM") as ps_g,
            tc.tile_pool(name="ps_o", bufs=2, space="PSUM") as ps_o,
        ):
            bias_t = cst.tile([128, 1], F32, tag="bias")
            nc.vector.memset(bias_t, act_bias)
            edt = cst.tile([128, 265], F16, tag="ed")

            loads = {}

            def emit_loads(b):
                xTt = xtp.tile([128, T], F16, tag="xT")
                if b == 0:
                    # split first load so B1's QK (needs cols < 512 only)
                    # starts one half-transfer earlier
                    nc.sync.dma_start(out=xTt[:, 0:512], in_=xT[b][:, 0:512])
                    nc.sync.dma_start(out=xTt[:, 512:T], in_=xT[b][:, 512:T])
                else:
                    nc.sync.dma_start(out=xTt, in_=xT[b])
                xm8t = xmp.tile([128, 4, 2, D + 1], F8, tag="xm8")
                nc.gpsimd.dma_start(out=xm8t, in_=xm8[b])
                xm16t = xmp.tile([128, NT, D + 1], F16, tag="xm16")
                nc.gpsimd.dma_start(out=xm16t, in_=xm16[b])
                loads[b] = (xTt, xm8t, xm16t)

            state = {}

            def emit_group(b, g):
                xTt = loads[b][0]
                if g["name"] == "B1":
                    E16 = e16p.tile([128, 2944], F16, tag="E16")
                    E8 = e8p.tile([128, 6, T], F8, tag="E8")
                    y16 = yp.tile([128, T + 2 * NT], F16, tag="y16")
                    state[b] = (E16, E8, y16)
                E16, E8, y16 = state[b]
                pS = ps_g.tile([128, 1024], F32, tag="pS")
                for s, pcol, t0, w in g["qk"]:
                    nc.tensor.matmul(
                        pS[:, pcol : pcol + w],
                        lhsT=xTt[:, s * 128 : (s + 1) * 128],
                        rhs=xTt[:, t0 : t0 + w],
                        start=True,
                        stop=True,
                    )
                for dst, where, plo, w in g["act"]:
                    if dst == "E16":
                        out = E16[:, where : where + w]
                    else:
                        pl, c0 = where
                        out = E8[:, pl, c0 : c0 + w]
                    nc.scalar.activation(
                        out=out, in_=pS[:, plo : plo + w], func=Exp, bias=bias_t,
                        scale=1.0,
                    )
                for dst, (pl, c0), plo, w in g["schraud"]:
                    nc.vector.tensor_scalar(
                        out=E8[:, pl, c0 : c0 + w].bitcast(I8),
                        in0=pS[:, plo : plo + w],
                        scalar1=C8,
                        scalar2=b8,
                        op0=Mult,
                        op1=Add,
                    )
                for s in g["band"]:
                    flat, edo, w = _band_window(s)
                    eng = nc.gpsimd if s in POOL_BAND else nc.vector
                    eng.tensor_mul(
                        E16[:, flat : flat + w],
                        E16[:, flat : flat + w],
                        edt[:, edo : edo + w],
                    )

            pvpair = {}

            def emit_pv(b, tt):
                _, xm8t, xm16t = loads[b]
                E16, E8, y16 = state[b]
                if tt % 2 == 0:
                    pOp = ps_o.tile([128, 2, D + 1], F32, tag="pO")
                    pvpair[b] = pOp
                pO = pvpair[b][:, tt % 2, :]
                n_far = max(0, tt - 1)
                mms = []
                for pr in range(n_far // 2):
                    mms.append(
                        dict(
                            lhsT=E8[:, 2 * pr : 2 * pr + 2, tt * 128 : (tt + 1) * 128],
                            rhs=xm8t[:, pr, :, :],
                            perf_mode=DR,
                        )
                    )
                if n_far % 2:
                    s = n_far - 1
                    mms.append(
                        dict(
                            lhsT=E8[:, s, tt * 128 : (tt + 1) * 128],
                            rhs=xm8t[:, s // 2, s % 2, :],
                        )
                    )
                for s in (tt - 1, tt, tt + 1):
                    if 0 <= s <= 7:
                        flat = s * 384 + (tt - s + 1) * 128
                        mms.append(
                            dict(lhsT=E16[:, flat : flat + 128], rhs=xm16t[:, s, :])
                        )
                for i, mm in enumerate(mms):
                    nc.tensor.matmul(
                        pO,
                        start=(i == 0),
                        stop=(i == len(mms) - 1),
                        skip_group_check=True,
                        **mm,
                    )
                if tt in EPI_ACT:
                    nc.scalar.activation(
                        out=y16[:, tt * 128 : (tt + 1) * 128], in_=pO[:, 0:D],
                        func=Copy, bias=0.0, scale=0.0625,
                    )
                else:
                    nc.vector.tensor_scalar_mul(
                        y16[:, tt * 128 : (tt + 1) * 128], pO[:, 0:D], 0.0625
                    )
                if tt % 2 == 1:  # both chains of the pair done -> copy r/16
                    q = tt // 2
                    nc.vector.tensor_scalar_mul(
                        y16[:, T + 2 * q : T + 2 * q + 2],
                        pvpair[b][:, :, D],
                        0.0625,
                    )
                if b == BPC - 1:
                    # last batch: flush y in small chunks on the idle SP queue
                    # right after each pair of tiles, shrinking the tail
                    if tt in (1, 3, 5):
                        c0, c1 = (tt - 1) * 128, (tt + 1) * 128
                        nc.sync.dma_start(out=y[b][:, c0:c1], in_=y16[:, c0:c1])
                    elif tt == 7:
                        c1 = T + 2 * NT
                        nc.sync.dma_start(out=y[b][:, 768:c1], in_=y16[:, 768:c1])
                        state.pop(b)
                        loads.pop(b)
                elif tt == NT - 1:
                    nc.gpsimd.dma_start(out=y[b], in_=y16)
                    state.pop(b)
                    loads.pop(b)

            emit_loads(0)
            nc.sync.dma_start(out=edt, in_=ed[:, :])
            emit_loads(1)
            for b in range(BPC):
                if b + 2 < BPC:
                    emit_loads(b + 2)
                for g in GROUPS:
                    emit_group(b, g)
                    for tt in PV_AFTER[g["name"]]:
                        emit_pv(b, tt)

    nc.compile()
    return nc


_NC = None
_NC_BIAS = None


def _get_module(act_bias=None):
    global _NC, _NC_BIAS
    if act_bias is None:
        assert _NC is not None, "module not built yet"
        return _NC
    if _NC is None or _NC_BIAS != act_bias:
        _NC = _build_module(act_bias)
        _NC_BIAS = act_bias
    return _NC


def prepare_inputs(x, mask):
    """Host-side prep: per-core input dicts (cheap O(B*T*D) / O(T) work)."""
    x = np.asarray(x, dtype=np.float32)
    m = np.asarray(mask).astype(np.float32)

    maxn2 = float((x * x).sum(axis=2).max())
    act_bias = FP8_MAX_ARG - maxn2  # A = 10 + EXP_BIAS

    x16 = x.astype(np.float16)
    xT16 = np.ascontiguousarray(x16.transpose(0, 2, 1))  # [B, 128, T]

    xmf = np.concatenate([x * m[:, :, None], m[:, :, None]], axis=2)  # [B,T,129]
    xm8 = np.ascontiguousarray(
        xmf.reshape(B, 4, 2, 128, D + 1).transpose(0, 3, 1, 2, 4)
    ).astype(ml_dtypes.float8_e4m3)
    xm16 = np.ascontiguousarray(
        xmf.reshape(B, NT, 128, D + 1).transpose(0, 2, 1, 3)
    ).astype(np.float16)

    jj = np.arange(265)[None, :]
    pp = np.arange(128)[:, None]
    ed = np.exp(
        np.minimum(jj - 128 - pp, DIST_CAP).astype(np.float64) - DIST_CAP
    ).astype(np.float16)

    in_maps = []
    for c in range(NCORES):
        sl = slice(c * BPC, (c + 1) * BPC)
        in_maps.append(
            {
                "xT": np.ascontiguousarray(xT16[sl]),
                "xm8": np.ascontiguousarray(xm8[sl]),
                "xm16": np.ascontiguousarray(xm16[sl]),
                "ed": ed,
            }
        )
    return in_maps, act_bias


def kernel(x, mask):
    in_maps, act_bias = prepare_inputs(x, mask)
    nc = _get_module(act_bias)
    res = run_bass_kernel_spmd(nc, in_maps, core_ids=list(range(NCORES)))
    yv = np.concatenate([res.results[c]["y"] for c in range(NCORES)], axis=0)
    yv = yv.astype(np.float32)
    # acc[b, p, tt*128+d] -> [b, tt*128+p, d]; r packed at cols T + 2q (+pair)
    acc = yv[:, :, 0:T].reshape(B, 128, NT, D).transpose(0, 2, 1, 3).reshape(B, T, D)
    r = yv[:, :, T : T + 2 * NT].reshape(B, 128, NT).transpose(0, 2, 1).reshape(B, T)
    m = np.asarray(mask).astype(np.float32)
    out = acc / r[:, :, None] * m[:, :, None]
    return np.ascontiguousarray(out).astype(np.float32)


# revision 24
# speedup vs baseline: 1.0612x; 1.0612x over previous
"""IntraSentenceAttention Trainium2 kernel (v2).

out[b,t,:] = sum_s P[t,s] x[b,s,:],  P = row-normalized exp(x@x^T + min(t-s,10))
masked by m_t, m_s (no max-subtract).

Strategy (8 NeuronCores, data-parallel over batch, 4 batches/core):
  - Scores computed transposed E^T[s,t] so PV needs no transpose. Tiles with
    t_tile <= s_tile-2 underflow to 0 and are skipped: 43 of 64 survive.
  - dist trick: exp(min(t-s,10)) == e^10 everywhere except a 265-diagonal band
    (t-s < 10). Fold e^10 into the ACT exp bias (uniform A = 10 + EXP_BIAS);
    multiply only the band window by a resident [128,265] master tile holding
    exp(min(t-s,10)-10)  (value 1.0 at the window edge, ->0 below diagonal).
  - Region split per s-row: band tiles {s-1,s,s+1} exp'd to fp16; far tiles
    (t_tile >= s_tile+2, where dist==10 exactly) exp'd to fp8e4 and consumed
    by DoubleRow fp8 matmuls (2 s-tiles per pass, 0.5 cyc/col) in PV.
  - exp work split ACT/DVE: most slices on ACT (func=Exp); a tuned share on
    DVE via Schraudolph (i8 = S*8*log2e + b, bitcast int8->fp8e4). The +-3-7%
    elementwise noise averages out across ~512 softmax terms (verified
    end-to-end rel err << 2e-2 tolerance).
  - PV accumulates [128, D+1] per 128-row tile; col D (ones in xm) gives the
    row sum r. Epilogue = ONE DVE tensor_scalar: out = (acc / r) * m_t -> fp16.
    (eps is negligible: r >= ~1e3 with these biases.)
  - All DMAs have >=512B contiguous descriptors via host-side prepacking;
    issue split across SP and Pool queues to avoid sequencer serialization.
"""

import sys

sys.path.insert(0, "/opt/trn_rl_repo")

import numpy as np
import ml_dtypes

import concourse.bacc as bacc
import concourse.tile as tile
from concourse import mybir
from concourse.bass_utils import run_bass_kernel_spmd

B, T, D = 32, 1024, 128
NCORES = 8
BPC = B // NCORES
NT = T // 128
DIST_CAP = 10

F32 = mybir.dt.float32
F16 = mybir.dt.float16
F8 = mybir.dt.float8e4
I16 = mybir.dt.int16
I8 = mybir.dt.int8
DR = mybir.MatmulPerfMode.DoubleRow

LOG2E = float(np.log2(np.e))
C8 = 8.0 * LOG2E          # Schraudolph fp8 scale
SIG8 = -0.46              # Schraudolph shift (centers the log-error)

# exp argument cap: largest score is bounded by max ||x_t||^2 (Cauchy-Schwarz);
# A = 10 + EXP_BIAS chosen so exp(S + A) <= 224 < fp8e4m3 max.
FP8_MAX_ARG = float(np.log(224.0))

# ---- static schedule tables (per batch) -----------------------------------
# s-row widths: row s covers t in [max(0,s-1)*128, 1024)
# Groups: B* produce fp16 band tiles {s-1,s,s+1}; F* produce fp8 far tiles
# (t_tile >= s+2). Each QK chunk: (s, pcol, t0, w), pcol never crossing a
# 512-col PSUM bank boundary.
GROUPS = [
    dict(  # band s=0,1,2 -> E16 flat [128, 1152)
        name="B1", width=1024,
        qk=[(0, 0, 0, 256), (1, 256, 0, 256), (1, 512, 256, 128),
            (2, 640, 128, 384)],
        act=[("E16", 128, 0, 1024)],
        schraud=[],
        band=[0, 1, 2],
    ),
    dict(  # far s=0
        name="F0", width=768,
        qk=[(0, 0, 256, 512), (0, 512, 768, 256)],
        act=[("E8", (0, 256), 0, 768)],
        schraud=[],
        band=[],
    ),
    dict(  # band s=3,4 -> E16 flat [1152, 1920)
        name="B2", width=768,
        qk=[(3, 0, 256, 384), (4, 384, 384, 128), (4, 512, 512, 256)],
        act=[("E16", 1152, 0, 768)],
        schraud=[],
        band=[3, 4],
    ),
    dict(  # far s=1  (Schraudolph on DVE)
        name="F1b", width=640,
        qk=[(1, 0, 384, 512), (1, 512, 896, 128)],
        act=[],
        schraud=[("E8", (1, 384), 0, 640)],
        band=[],
    ),
    dict(  # far s=2,3
        name="F2a", width=896,
        qk=[(2, 0, 512, 512), (3, 512, 640, 384)],
        act=[("E8", (2, 512), 0, 512), ("E8", (3, 640), 512, 384)],
        schraud=[],
        band=[],
    ),
    dict(  # band s=5,6,7 -> E16 flat [1920, 2944)
        name="B3", width=1024,
        qk=[(5, 0, 512, 384), (6, 384, 640, 128), (6, 512, 768, 256),
            (7, 768, 768, 256)],
        act=[("E16", 1920, 0, 1024)],
        schraud=[],
        band=[5, 6, 7],
    ),
    dict(  # far s=4,5  (Schraudolph on DVE)
        name="F2b", width=384,
        qk=[(4, 0, 768, 256), (5, 256, 896, 128)],
        act=[],
        schraud=[("E8", (4, 768), 0, 256), ("E8", (5, 896), 256, 128)],
        band=[],
    ),
]
# PV tiles emitted after each group (deps satisfied at that point)
PV_AFTER = {"B1": [0, 1], "F0": [], "B2": [2], "F1b": [3], "F2a": [],
            "B3": [4, 5], "F2b": [6, 7]}
POOL_BAND = ()          # band-mul rows offloaded to the (idle) Pool engine
EPI_ACT = (2,)          # PV tiles whose acc-copy runs on ACT instead of DVE

# band-mul window (E16 flat offset, width) per s: j in [0,265) except edges
def _band_window(s):
    j0 = 128 if s == 0 else 0
    j1 = min(265, T - (s - 1) * 128)
    return s * 384 + j0, j0, j1 - j0  # (flat offset, ed offset, width)


def _build_module(act_bias):
    nc = bacc.Bacc("TRN2", target_bir_lowering=False, debug=False, num_devices=NCORES)
    xT = nc.declare_dram_parameter("xT", [BPC, 128, T], F16, isOutput=False)
    xm8 = nc.declare_dram_parameter("xm8", [BPC, 128, 4, 2, D + 1], F8, isOutput=False)
    xm16 = nc.declare_dram_parameter("xm16", [BPC, 128, NT, D + 1], F16, isOutput=False)
    ed = nc.declare_dram_parameter("ed", [128, 265], F16, isOutput=False)
    # y cols [0,1024) = acc/16 per (tt,d); cols [1024,1040) = r/16 per pair-col
    y = nc.declare_dram_parameter("y", [BPC, 128, T + 2 * NT], F16, isOutput=True)

    Exp = mybir.ActivationFunctionType.Exp
    Copy = mybir.ActivationFunctionType.Copy
    Mult = mybir.AluOpType.mult
    Add = mybir.AluOpType.add
    Div = mybir.AluOpType.divide
    b8 = act_bias * C8 + 56.0 + SIG8

    with tile.TileContext(nc) as tc:
        with (
            tc.tile_pool(name="cst", bufs=1) as cst,
            tc.tile_pool(name="xtp", bufs=3) as xtp,
            tc.tile_pool(name="xmp", bufs=3) as xmp,
            tc.tile_pool(name="e16p", bufs=2) as e16p,
            tc.tile_pool(name="e8p", bufs=2) as e8p,
            tc.tile_pool(name="yp", bufs=2) as yp,
            tc.tile_pool(name="ps_g", bufs=3, space="PSUM") as ps_g,
            tc.tile_pool(name="ps_o", bufs=2, space="PSUM") as ps_o,
        ):
            bias_t = cst.tile([128, 1], F32, tag="bias")
            nc.vector.memset(bias_t, act_bias)
            edt = cst.tile([128, 265], F16, tag="ed")
            # tiny matmul at t~0 starts the PE p-state ramp so real QK
            # matmuls (first data lands ~3us) run at full clock
            warm_ps = ps_g.tile([128, 1024], F32, tag="pS")
            nc.tensor.matmul(
                warm_ps[0:1, 0:1], lhsT=bias_t[:, 0:1], rhs=bias_t[:, 0:1],
                start=True, stop=True,
            )

            loads = {}

            def emit_loads(b):
                xTt = xtp.tile([128, T], F16, tag="xT")
                if b == 0:
                    # split first load so B1's QK (needs cols < 512 only)
                    # starts one half-transfer earlier
                    nc.sync.dma_start(out=xTt[:, 0:512], in_=xT[b][:, 0:512])
                    nc.sync.dma_start(out=xTt[:, 512:T], in_=xT[b][:, 512:T])
                else:
                    nc.sync.dma_start(out=xTt, in_=xT[b])
                xm8t = xmp.tile([128, 4, 2, D + 1], F8, tag="xm8")
                nc.gpsimd.dma_start(out=xm8t, in_=xm8[b])
                xm16t = xmp.tile([128, NT, D + 1], F16, tag="xm16")
                nc.sync.dma_start(out=xm16t, in_=xm16[b])
                loads[b] = (xTt, xm8t, xm16t)

            state = {}

            def emit_group(b, g):
                xTt = loads[b][0]
                if g["name"] == "B1":
                    E16 = e16p.tile([128, 2944], F16, tag="E16")
                    E8 = e8p.tile([128, 6, T], F8, tag="E8")
                    y16 = yp.tile([128, T + 2 * NT], F16, tag="y16")
                    state[b] = (E16, E8, y16)
                E16, E8, y16 = state[b]
                pS = ps_g.tile([128, 1024], F32, tag="pS")
                for s, pcol, t0, w in g["qk"]:
                    nc.tensor.matmul(
                        pS[:, pcol : pcol + w],
                        lhsT=xTt[:, s * 128 : (s + 1) * 128],
                        rhs=xTt[:, t0 : t0 + w],
                        start=True,
                        stop=True,
                    )
                for dst, where, plo, w in g["act"]:
                    if dst == "E16":
                        out = E16[:, where : where + w]
                    else:
                        pl, c0 = where
                        out = E8[:, pl, c0 : c0 + w]
                    nc.scalar.activation(
                        out=out, in_=pS[:, plo : plo + w], func=Exp, bias=bias_t,
                        scale=1.0,
                    )
                for dst, (pl, c0), plo, w in g["schraud"]:
                    nc.vector.tensor_scalar(
                        out=E8[:, pl, c0 : c0 + w].bitcast(I8),
                        in0=pS[:, plo : plo + w],
                        scalar1=C8,
                        scalar2=b8,
                        op0=Mult,
                        op1=Add,
                    )
                for s in g["band"]:
                    flat, edo, w = _band_window(s)
                    eng = nc.gpsimd if s in POOL_BAND else nc.vector
                    eng.tensor_mul(
                        E16[:, flat : flat + w],
                        E16[:, flat : flat + w],
                        edt[:, edo : edo + w],
                    )

            pvpair = {}

            def emit_pv(b, tt):
                _, xm8t, xm16t = loads[b]
                E16, E8, y16 = state[b]
                if tt % 2 == 0:
                    pOp = ps_o.tile([128, 2, D + 1], F32, tag="pO")
                    pvpair[b] = pOp
                pO = pvpair[b][:, tt % 2, :]
                n_far = max(0, tt - 1)
                mms = []
                for pr in range(n_far // 2):
                    mms.append(
                        dict(
                            lhsT=E8[:, 2 * pr : 2 * pr + 2, tt * 128 : (tt + 1) * 128],
                            rhs=xm8t[:, pr, :, :],
                            perf_mode=DR,
                        )
                    )
                if n_far % 2:
                    s = n_far - 1
                    mms.append(
                        dict(
                            lhsT=E8[:, s, tt * 128 : (tt + 1) * 128],
                            rhs=xm8t[:, s // 2, s % 2, :],
                        )
                    )
                for s in (tt - 1, tt, tt + 1):
                    if 0 <= s <= 7:
                        flat = s * 384 + (tt - s + 1) * 128
                        mms.append(
                            dict(lhsT=E16[:, flat : flat + 128], rhs=xm16t[:, s, :])
                        )
                for i, mm in enumerate(mms):
                    nc.tensor.matmul(
                        pO,
                        start=(i == 0),
                        stop=(i == len(mms) - 1),
                        skip_group_check=True,
                        **mm,
                    )
                if tt in EPI_ACT:
                    nc.scalar.activation(
                        out=y16[:, tt * 128 : (tt + 1) * 128], in_=pO[:, 0:D],
                        func=Copy, bias=0.0, scale=0.0625,
                    )
                else:
                    nc.vector.tensor_scalar_mul(
                        y16[:, tt * 128 : (tt + 1) * 128], pO[:, 0:D], 0.0625
                    )
                if tt % 2 == 1:  # both chains of the pair done -> copy r/16
                    q = tt // 2
                    nc.vector.tensor_scalar_mul(
                        y16[:, T + 2 * q : T + 2 * q + 2],
                        pvpair[b][:, :, D],
                        0.0625,
                    )
                if b == BPC - 1:
                    # last batch: flush y in small chunks on the idle SP queue
                    # right after each pair of tiles, shrinking the tail
                    if tt in (1, 3, 5):
                        c0, c1 = (tt - 1) * 128, (tt + 1) * 128
                        nc.sync.dma_start(out=y[b][:, c0:c1], in_=y16[:, c0:c1])
                    elif tt == 7:
                        c1 = T + 2 * NT
                        nc.sync.dma_start(out=y[b][:, 768:c1], in_=y16[:, 768:c1])
                        state.pop(b)
                        loads.pop(b)
                elif tt == NT - 1:
                    nc.gpsimd.dma_start(out=y[b], in_=y16)
                    state.pop(b)
                    loads.pop(b)

            emit_loads(0)
            nc.sync.dma_start(out=edt, in_=ed[:, :])
            emit_loads(1)
            for b in range(BPC):
                if b + 2 < BPC:
                    emit_loads(b + 2)
                for g in GROUPS:
                    emit_group(b, g)
                    for tt in PV_AFTER[g["name"]]:
                        emit_pv(b, tt)

    nc.compile()
    return nc


_NC = None
_NC_BIAS = None


def _get_module(act_bias=None):
    global _NC, _NC_BIAS
    if act_bias is None:
        assert _NC is not None, "module not built yet"
        return _NC
    if _NC is None or _NC_BIAS != act_bias:
        _NC = _build_module(act_bias)
        _NC_BIAS = act_bias
    return _NC


def prepare_inputs(x, mask):
    """Host-side prep: per-core input dicts (cheap O(B*T*D) / O(T) work)."""
    x = np.asarray(x, dtype=np.float32)
    m = np.asarray(mask).astype(np.float32)

    maxn2 = float((x * x).sum(axis=2).max())
    act_bias = FP8_MAX_ARG - maxn2  # A = 10 + EXP_BIAS

    x16 = x.astype(np.float16)
    xT16 = np.ascontiguousarray(x16.transpose(0, 2, 1))  # [B, 128, T]

    xmf = np.concatenate([x * m[:, :, None], m[:, :, None]], axis=2)  # [B,T,129]
    xm8 = np.ascontiguousarray(
        xmf.reshape(B, 4, 2, 128, D + 1).transpose(0, 3, 1, 2, 4)
    ).astype(ml_dtypes.float8_e4m3)
    xm16 = np.ascontiguousarray(
        xmf.reshape(B, NT, 128, D + 1).transpose(0, 2, 1, 3)
    ).astype(np.float16)

    jj = np.arange(265)[None, :]
    pp = np.arange(128)[:, None]
    ed = np.exp(
        np.minimum(jj - 128 - pp, DIST_CAP).astype(np.float64) - DIST_CAP
    ).astype(np.float16)

    in_maps = []
    for c in range(NCORES):
        sl = slice(c * BPC, (c + 1) * BPC)
        in_maps.append(
            {
                "xT": np.ascontiguousarray(xT16[sl]),
                "xm8": np.ascontiguousarray(xm8[sl]),
                "xm16": np.ascontiguousarray(xm16[sl]),
                "ed": ed,
            }
        )
    return in_maps, act_bias


def kernel(x, mask):
    in_maps, act_bias = prepare_inputs(x, mask)
    nc = _get_module(act_bias)
    res = run_bass_kernel_spmd(nc, in_maps, core_ids=list(range(NCORES)))
    yv = np.concatenate([res.results[c]["y"] for c in range(NCORES)], axis=0)
    yv = yv.astype(np.float32)
    # acc[b, p, tt*128+d] -> [b, tt*128+p, d]; r packed at cols T + 2q (+pair)
    acc = yv[:, :, 0:T].reshape(B, 128, NT, D).transpose(0, 2, 1, 3).reshape(B, T, D)
    r = yv[:, :, T : T + 2 * NT].reshape(B, 128, NT).transpose(0, 2, 1).reshape(B, T)
    m = np.asarray(mask).astype(np.float32)
    out = acc / r[:, :, None] * m[:, :, None]
    return np.ascontiguousarray(out).astype(np.float32)


# revision 27
# speedup vs baseline: 1.1166x; 1.0522x over previous
"""IntraSentenceAttention Trainium2 kernel (v2).

out[b,t,:] = sum_s P[t,s] x[b,s,:],  P = row-normalized exp(x@x^T + min(t-s,10))
masked by m_t, m_s (no max-subtract).

Strategy (8 NeuronCores, data-parallel over batch, 4 batches/core):
  - Scores computed transposed E^T[s,t] so PV needs no transpose. Tiles with
    t_tile <= s_tile-2 underflow to 0 and are skipped: 43 of 64 survive.
  - dist trick: exp(min(t-s,10)) == e^10 everywhere except a 265-diagonal band
    (t-s < 10). Fold e^10 into the ACT exp bias (uniform A = 10 + EXP_BIAS);
    multiply only the band window by a resident [128,265] master tile holding
    exp(min(t-s,10)-10)  (value 1.0 at the window edge, ->0 below diagonal).
  - Region split per s-row: band tiles {s-1,s,s+1} exp'd to fp16; far tiles
    (t_tile >= s_tile+2, where dist==10 exactly) exp'd to fp8e4 and consumed
    by DoubleRow fp8 matmuls (2 s-tiles per pass, 0.5 cyc/col) in PV.
  - exp work split ACT/DVE: most slices on ACT (func=Exp); a tuned share on
    DVE via Schraudolph (i8 = S*8*log2e + b, bitcast int8->fp8e4). The +-3-7%
    elementwise noise averages out across ~512 softmax terms (verified
    end-to-end rel err << 2e-2 tolerance).
  - PV accumulates [128, D+1] per 128-row tile; col D (ones in xm) gives the
    row sum r. Epilogue = ONE DVE tensor_scalar: out = (acc / r) * m_t -> fp16.
    (eps is negligible: r >= ~1e3 with these biases.)
  - All DMAs have >=512B contiguous descriptors via host-side prepacking;
    issue split across SP and Pool queues to avoid sequencer serialization.
"""

import sys

sys.path.insert(0, "/opt/trn_rl_repo")

import numpy as np
import ml_dtypes

import concourse.bacc as bacc
import concourse.tile as tile
from concourse import mybir
from concourse.bass_utils import run_bass_kernel_spmd

B, T, D = 32, 1024, 128
NCORES = 8
BPC = B // NCORES
NT = T // 128
DIST_CAP = 10

F32 = mybir.dt.float32
F16 = mybir.dt.float16
F8 = mybir.dt.float8e4
I16 = mybir.dt.int16
I8 = mybir.dt.int8
DR = mybir.MatmulPerfMode.DoubleRow

LOG2E = float(np.log2(np.e))
C8 = 8.0 * LOG2E          # Schraudolph fp8 scale
SIG8 = -0.46              # Schraudolph shift (centers the log-error)

# exp argument cap: largest score is bounded by max ||x_t||^2 (Cauchy-Schwarz);
# A = 10 + EXP_BIAS chosen so exp(S + A) <= 224 < fp8e4m3 max.
FP8_MAX_ARG = float(np.log(224.0))

# ---- static schedule tables (per batch) -----------------------------------
# s-row widths: row s covers t in [max(0,s-1)*128, 1024)
# Groups: B* produce fp16 band tiles {s-1,s,s+1}; F* produce fp8 far tiles
# (t_tile >= s+2). Each QK chunk: (s, pcol, t0, w), pcol never crossing a
# 512-col PSUM bank boundary.
GROUPS = [
    dict(  # band s=0,1,2 -> E16 flat [128, 1152)
        name="B1", width=1024,
        qk=[(0, 0, 0, 256), (1, 256, 0, 256), (1, 512, 256, 128),
            (2, 640, 128, 384)],
        act=[("E16", 128, 0, 1024)],
        schraud=[],
        band=[0, 1, 2],
    ),
    dict(  # far s=0
        name="F0", width=768,
        qk=[(0, 0, 256, 512), (0, 512, 768, 256)],
        act=[("E8", (0, 256), 0, 768)],
        schraud=[],
        band=[],
    ),
    dict(  # band s=3,4 -> E16 flat [1152, 1920)
        name="B2", width=768,
        qk=[(3, 0, 256, 384), (4, 384, 384, 128), (4, 512, 512, 256)],
        act=[("E16", 1152, 0, 768)],
        schraud=[],
        band=[3, 4],
    ),
    dict(  # far s=1  (Schraudolph on DVE)
        name="F1b", width=640,
        qk=[(1, 0, 384, 512), (1, 512, 896, 128)],
        act=[],
        schraud=[("E8", (1, 384), 0, 640)],
        band=[],
    ),
    dict(  # far s=2,3
        name="F2a", width=896,
        qk=[(2, 0, 512, 512), (3, 512, 640, 384)],
        act=[("E8", (2, 512), 0, 512), ("E8", (3, 640), 512, 384)],
        schraud=[],
        band=[],
    ),
    dict(  # band s=5,6,7 -> E16 flat [1920, 2944)
        name="B3", width=1024,
        qk=[(5, 0, 512, 384), (6, 384, 640, 128), (6, 512, 768, 256),
            (7, 768, 768, 256)],
        act=[("E16", 1920, 0, 1024)],
        schraud=[],
        band=[5, 6, 7],
    ),
    dict(  # far s=4,5  (Schraudolph on DVE)
        name="F2b", width=384,
        qk=[(4, 0, 768, 256), (5, 256, 896, 128)],
        act=[],
        schraud=[("E8", (4, 768), 0, 256), ("E8", (5, 896), 256, 128)],
        band=[],
    ),
]
# PV tiles emitted after each group (deps satisfied at that point)
PV_AFTER = {"B1": [0, 1], "F0": [], "B2": [2], "F1b": [3], "F2a": [],
            "B3": [4, 5], "F2b": [6, 7]}
POOL_BAND = ()          # band-mul rows offloaded to the (idle) Pool engine
EPI_ACT = (2,)          # PV tiles whose acc-copy runs on ACT instead of DVE

# band-mul window (E16 flat offset, width) per s: j in [0,265) except edges
def _band_window(s):
    j0 = 128 if s == 0 else 0
    j1 = min(265, T - (s - 1) * 128)
    return s * 384 + j0, j0, j1 - j0  # (flat offset, ed offset, width)


def _build_module(act_bias):
    nc = bacc.Bacc("TRN2", target_bir_lowering=False, debug=False, num_devices=NCORES)
    xT = nc.declare_dram_parameter("xT", [BPC, 128, T], F16, isOutput=False)
    xm8 = nc.declare_dram_parameter("xm8", [BPC, 128, 4, 2, D + 1], F8, isOutput=False)
    xm16 = nc.declare_dram_parameter("xm16", [BPC, 128, NT, D + 1], F16, isOutput=False)
    ed = nc.declare_dram_parameter("ed", [128, 265], F16, isOutput=False)
    # y cols [0,1024) = acc/16 per (tt,d); cols [1024,1032) = r/16 per tt
    y = nc.declare_dram_parameter("y", [BPC, 128, T + NT], F16, isOutput=True)

    Exp = mybir.ActivationFunctionType.Exp
    Copy = mybir.ActivationFunctionType.Copy
    Mult = mybir.AluOpType.mult
    Add = mybir.AluOpType.add
    Div = mybir.AluOpType.divide
    b8 = act_bias * C8 + 56.0 + SIG8

    with tile.TileContext(nc) as tc:
        with (
            tc.tile_pool(name="cst", bufs=1) as cst,
            tc.tile_pool(name="xtp", bufs=3) as xtp,
            tc.tile_pool(name="xmp", bufs=3) as xmp,
            tc.tile_pool(name="e16p", bufs=2) as e16p,
            tc.tile_pool(name="e8p", bufs=2) as e8p,
            tc.tile_pool(name="yp", bufs=2) as yp,
            tc.tile_pool(name="ps_g", bufs=3, space="PSUM") as ps_g,
            tc.tile_pool(name="ps_o", bufs=2, space="PSUM") as ps_o,
        ):
            bias_t = cst.tile([128, 1], F32, tag="bias")
            nc.vector.memset(bias_t, act_bias)
            edt = cst.tile([128, 265], F16, tag="ed")
            # tiny matmul at t~0 starts the PE p-state ramp so real QK
            # matmuls (first data lands ~3us) run at full clock
            warm_ps = ps_g.tile([128, 1024], F32, tag="pS")
            nc.tensor.matmul(
                warm_ps[0:1, 0:1], lhsT=bias_t[:, 0:1], rhs=bias_t[:, 0:1],
                start=True, stop=True,
            )

            loads = {}

            def emit_loads(b):
                xTt = xtp.tile([128, T], F16, tag="xT")
                if b == 0:
                    # split first load so B1's QK (needs cols < 512 only)
                    # starts one half-transfer earlier
                    nc.sync.dma_start(out=xTt[:, 0:512], in_=xT[b][:, 0:512])
                    nc.sync.dma_start(out=xTt[:, 512:T], in_=xT[b][:, 512:T])
                else:
                    nc.sync.dma_start(out=xTt, in_=xT[b])
                xm8t = xmp.tile([128, 4, 2, D + 1], F8, tag="xm8")
                nc.gpsimd.dma_start(out=xm8t, in_=xm8[b])
                xm16t = xmp.tile([128, NT, D + 1], F16, tag="xm16")
                nc.sync.dma_start(out=xm16t, in_=xm16[b])
                loads[b] = (xTt, xm8t, xm16t)

            state = {}

            def emit_group(b, g):
                xTt = loads[b][0]
                if g["name"] == "B1":
                    E16 = e16p.tile([128, 2944], F16, tag="E16")
                    E8 = e8p.tile([128, 6, T], F8, tag="E8")
                    y16 = yp.tile([128, T + NT], F16, tag="y16")
                    state[b] = (E16, E8, y16)
                E16, E8, y16 = state[b]
                pS = ps_g.tile([128, 1024], F32, tag="pS")
                for s, pcol, t0, w in g["qk"]:
                    nc.tensor.matmul(
                        pS[:, pcol : pcol + w],
                        lhsT=xTt[:, s * 128 : (s + 1) * 128],
                        rhs=xTt[:, t0 : t0 + w],
                        start=True,
                        stop=True,
                    )
                for dst, where, plo, w in g["act"]:
                    if dst == "E16":
                        out = E16[:, where : where + w]
                    else:
                        pl, c0 = where
                        out = E8[:, pl, c0 : c0 + w]
                    nc.scalar.activation(
                        out=out, in_=pS[:, plo : plo + w], func=Exp, bias=bias_t,
                        scale=1.0,
                    )
                for dst, (pl, c0), plo, w in g["schraud"]:
                    nc.vector.tensor_scalar(
                        out=E8[:, pl, c0 : c0 + w].bitcast(I8),
                        in0=pS[:, plo : plo + w],
                        scalar1=C8,
                        scalar2=b8,
                        op0=Mult,
                        op1=Add,
                    )
                for s in g["band"]:
                    flat, edo, w = _band_window(s)
                    eng = nc.gpsimd if s in POOL_BAND else nc.vector
                    eng.tensor_mul(
                        E16[:, flat : flat + w],
                        E16[:, flat : flat + w],
                        edt[:, edo : edo + w],
                    )

            pvpair = {}

            def emit_pv(b, tt):
                _, xm8t, xm16t = loads[b]
                E16, E8, y16 = state[b]
                pO = ps_o.tile([128, D + 1], F32, tag="pO")
                n_far = max(0, tt - 1)
                mms = []
                for pr in range(n_far // 2):
                    mms.append(
                        dict(
                            lhsT=E8[:, 2 * pr : 2 * pr + 2, tt * 128 : (tt + 1) * 128],
                            rhs=xm8t[:, pr, :, :],
                            perf_mode=DR,
                        )
                    )
                if n_far % 2:
                    s = n_far - 1
                    mms.append(
                        dict(
                            lhsT=E8[:, s, tt * 128 : (tt + 1) * 128],
                            rhs=xm8t[:, s // 2, s % 2, :],
                        )
                    )
                for s in (tt - 1, tt, tt + 1):
                    if 0 <= s <= 7:
                        flat = s * 384 + (tt - s + 1) * 128
                        mms.append(
                            dict(lhsT=E16[:, flat : flat + 128], rhs=xm16t[:, s, :])
                        )
                for i, mm in enumerate(mms):
                    nc.tensor.matmul(
                        pO,
                        start=(i == 0),
                        stop=(i == len(mms) - 1),
                        skip_group_check=True,
                        **mm,
                    )
                if tt in EPI_ACT:
                    nc.scalar.activation(
                        out=y16[:, tt * 128 : (tt + 1) * 128], in_=pO[:, 0:D],
                        func=Copy, bias=0.0, scale=0.0625,
                    )
                else:
                    nc.vector.tensor_scalar_mul(
                        y16[:, tt * 128 : (tt + 1) * 128], pO[:, 0:D], 0.0625
                    )
                nc.vector.tensor_scalar_mul(
                    y16[:, T + tt : T + tt + 1], pO[:, D : D + 1], 0.0625
                )
                if b == BPC - 1:
                    # last batch: flush y in small chunks on the idle SP queue
                    # right after each pair of tiles, shrinking the tail
                    if tt in (1, 3, 5):
                        c0, c1 = (tt - 1) * 128, (tt + 1) * 128
                        nc.sync.dma_start(out=y[b][:, c0:c1], in_=y16[:, c0:c1])
                    elif tt == 7:
                        c1 = T + NT
                        nc.sync.dma_start(out=y[b][:, 768:c1], in_=y16[:, 768:c1])
                        state.pop(b)
                        loads.pop(b)
                elif tt == NT - 1:
                    nc.gpsimd.dma_start(out=y[b], in_=y16)
                    state.pop(b)
                    loads.pop(b)

            emit_loads(0)
            nc.sync.dma_start(out=edt, in_=ed[:, :])
            emit_loads(1)
            for b in range(BPC):
                if b + 2 < BPC:
                    emit_loads(b + 2)
                for g in GROUPS:
                    emit_group(b, g)
                    for tt in PV_AFTER[g["name"]]:
                        emit_pv(b, tt)

    nc.compile()
    return nc


_NC = None
_NC_BIAS = None


def _get_module(act_bias=None):
    global _NC, _NC_BIAS
    if act_bias is None:
        assert _NC is not None, "module not built yet"
        return _NC
    if _NC is None or _NC_BIAS != act_bias:
        _NC = _build_module(act_bias)
        _NC_BIAS = act_bias
    return _NC


def prepare_inputs(x, mask):
    """Host-side prep: per-core input dicts (cheap O(B*T*D) / O(T) work)."""
    x = np.asarray(x, dtype=np.float32)
    m = np.asarray(mask).astype(np.float32)

    maxn2 = float((x * x).sum(axis=2).max())
    act_bias = FP8_MAX_ARG - maxn2  # A = 10 + EXP_BIAS

    x16 = x.astype(np.float16)
    xT16 = np.ascontiguousarray(x16.transpose(0, 2, 1))  # [B, 128, T]

    xmf = np.concatenate([x * m[:, :, None], m[:, :, None]], axis=2)  # [B,T,129]
    xm8 = np.ascontiguousarray(
        xmf.reshape(B, 4, 2, 128, D + 1).transpose(0, 3, 1, 2, 4)
    ).astype(ml_dtypes.float8_e4m3)
    xm16 = np.ascontiguousarray(
        xmf.reshape(B, NT, 128, D + 1).transpose(0, 2, 1, 3)
    ).astype(np.float16)

    jj = np.arange(265)[None, :]
    pp = np.arange(128)[:, None]
    ed = np.exp(
        np.minimum(jj - 128 - pp, DIST_CAP).astype(np.float64) - DIST_CAP
    ).astype(np.float16)

    in_maps = []
    for c in range(NCORES):
        sl = slice(c * BPC, (c + 1) * BPC)
        in_maps.append(
            {
                "xT": np.ascontiguousarray(xT16[sl]),
                "xm8": np.ascontiguousarray(xm8[sl]),
                "xm16": np.ascontiguousarray(xm16[sl]),
                "ed": ed,
            }
        )
    return in_maps, act_bias


def kernel(x, mask):
    in_maps, act_bias = prepare_inputs(x, mask)
    nc = _get_module(act_bias)
    res = run_bass_kernel_spmd(nc, in_maps, core_ids=list(range(NCORES)))
    yv = np.concatenate([res.results[c]["y"] for c in range(NCORES)], axis=0)
    yv = yv.astype(np.float32)
    # acc[b, p, tt*128+d] -> [b, tt*128+p, d]; r packed at cols T + 2q (+pair)
    acc = yv[:, :, 0:T].reshape(B, 128, NT, D).transpose(0, 2, 1, 3).reshape(B, T, D)
    r = yv[:, :, T : T + NT].reshape(B, 128, NT).transpose(0, 2, 1).reshape(B, T)
    m = np.asarray(mask).astype(np.float32)
    out = acc / r[:, :, None] * m[:, :, None]
    return np.ascontiguousarray(out).astype(np.float32)
